# revision 11
# baseline (speedup 1.0000x reference)
"""Trainium2 Bass kernel for a dense decoder layer (GQA attention + gated MLP).

Distribution (8 NeuronCores): DP=2 over batch x TP=4 megatron split.
  - cores 0-3: batch 0, model ranks 0-3; cores 4-7: batch 1.
  - wq/wk/wv column-sharded by heads (8 q / 2 kv heads per core), wo row-sharded;
    w_gate/w_val column-sharded, w_out row-sharded (FFN padded 10928->11264, 2816/core).
  - Two AllReduces (partial h_attn, partial h_dense) within each 4-core group.

On-device dataflow (per core, its full batch of 1024 tokens):
  S0  x -> rms_norm -> hn, PE-transposed to hnT [D, T] (fp32, SBUF resident)
  S1  QT/KT produced directly transposed (lhsT=W chunk, rhs=hnT), RoPE applied in
      T-layout (partition-shift DMA + DVE), V token-major; spilled to DRAM
  S2  attention per head, transposed-softmax: scoresT=[kv,q] via matmul, the 30*tanh
      soft-cap bounds logits so exp(l-30) needs no max pass; row sums via ones-matmul;
      causal handled by block skip + 0/1 mask tiles on partial blocks
  S3  wo row-shard matmul -> partial h_attn -> AllReduce#1
  S4  h1 = x + rms(h_attn)*s_post_attn; hn2 (pre-mlp scale folded into weights) cast
      bf16, PE-transposed to hn2T
  S5  gateT/valT (bf16 matmuls, fp32 psum), gelu_tanh * val -> actT bf16 (SBUF)
  S6  w_out row-shard matmul -> partial h_dense -> AllReduce#2
  S7  out = h1 + rms(h_dense)*s_post_mlp

All big matmuls use float32r (full PE rate, ~1e-3 worst-case rel err); the MLP runs
bf16 weights/activations with fp32 accumulation.
"""

import os

import numpy as np
import ml_dtypes

import concourse.bass as bass
import concourse.mybir as mybir
import concourse.tile as tile
from concourse import bacc
from concourse.bass_utils import run_bass_kernel_spmd
from concourse.masks import make_identity

F32 = mybir.dt.float32
F32R = mybir.dt.float32r
BF16 = mybir.dt.bfloat16
AF = mybir.ActivationFunctionType
ALU = mybir.AluOpType

# Problem dims
B, S, D = 2, 1024, 4096
NQ, NKV, HD = 32, 8, 128
FFN = 10928
ATTN_MULT = 0.08838834764831845
MAX_ATTN = 30.0
EPS = 1e-5
BASE = 10000.0

# Distribution
N_CORES = 8
TP = 4
RG = [[0, 1, 2, 3], [4, 5, 6, 7]]

# Per-core dims
T = S                      # tokens per core (its whole batch)
TC = T // 128              # 8 token chunks
TG = T // 512              # 2 token groups (matmul moving dim)
DC = D // 128              # 32 contraction chunks
DB = D // 512              # 8 output blocks
HQ = NQ // TP              # 8 q heads per core
HKV = NKV // TP            # 2 kv heads per core
GQ = NQ // NKV             # 4 q heads per kv head
FFN_PAD = 11264            # 22*128*4
F = FFN_PAD // TP          # 2816 ffn columns per core
FC = F // 128              # 22 ffn chunks
KC = S // 128              # 8 kv chunks


def _analyze_mask(mask_qk: np.ndarray):
    """Classify each (q-group-of-512, kv-chunk-of-128) block of mask[q, kv]:
    'skip' (all masked), 'full' (all visible), or a [128, 512] 0/1 tile in
    scoresT layout [kv, q]."""
    blocks = {}
    tiles = []
    for g in range(TG):
        for k in range(KC):
            blk = mask_qk[g * 512:(g + 1) * 512, k * 128:(k + 1) * 128]
            if not blk.any():
                blocks[(g, k)] = ("skip", -1)
            elif blk.all():
                blocks[(g, k)] = ("full", -1)
            else:
                blocks[(g, k)] = ("partial", len(tiles))
                tiles.append(blk.T.astype(np.float32))  # [kv 128, q 512]
    if not tiles:
        tiles.append(np.zeros((128, 512), np.float32))
    return blocks, np.stack(tiles)


def _build_nc(blocks, n_mask_tiles):
    nc = bacc.Bacc("TRN2", target_bir_lowering=False, debug=False,
                   num_devices=N_CORES)

    # Per-core external inputs
    x_d = nc.dram_tensor("x", [T, D], F32, kind="ExternalInput")
    wq_d = nc.dram_tensor("wq", [D, HQ * HD], F32R, kind="ExternalInput")
    wk_d = nc.dram_tensor("wk", [D, HKV * HD], F32R, kind="ExternalInput")
    wv_d = nc.dram_tensor("wv", [D, HKV * HD], F32R, kind="ExternalInput")
    wo_d = nc.dram_tensor("wo", [HQ * HD, D], F32R, kind="ExternalInput")
    wg_d = nc.dram_tensor("wg", [D, F], BF16, kind="ExternalInput")
    wv2_d = nc.dram_tensor("wv2", [D, F], BF16, kind="ExternalInput")
    wout_d = nc.dram_tensor("wout", [F, D], BF16, kind="ExternalInput")
    spa_d = nc.dram_tensor("sp_attn", [128, D], F32, kind="ExternalInput")
    spm_d = nc.dram_tensor("sp_mlp", [128, D], F32, kind="ExternalInput")
    cosT_d = nc.dram_tensor("cosT", [HD, T], F32, kind="ExternalInput")
    sinTn_d = nc.dram_tensor("sinTn", [HD, T], F32, kind="ExternalInput")
    dmask_d = nc.dram_tensor("dmask", [n_mask_tiles, 128, 512], F32,
                             kind="ExternalInput")
    out_d = nc.dram_tensor("out", [T, D], F32, kind="ExternalOutput")


    with tile.TileContext(nc) as tc:
        with (
            tc.tile_pool(name="dram", bufs=1, space="DRAM") as dram,
            tc.tile_pool(name="const", bufs=1) as const,
            tc.tile_pool(name="ms", bufs=4) as msp,
        ):
            # DRAM scratch
            qt_dram = dram.tile([HQ, HD, T], F32R)
            kt_dram = dram.tile([HKV, HD, T], F32R)
            v_dram = dram.tile([T, HKV * HD], F32R)
            ar1_in = dram.tile([T, D], F32)
            ar1_out = dram.tile([T, D], F32)
            ar2_in = dram.tile([T, D], F32)
            ar2_out = dram.tile([T, D], F32)
            h1_dram = dram.tile([T, D], F32)

            ident = const.tile([128, 128], F32)
            make_identity(nc, ident)
            ident_bf = const.tile([128, 128], BF16)
            nc.vector.tensor_copy(ident_bf[:], ident[:])
            ones_col_f = const.tile([128, 1], F32)
            nc.vector.memset(ones_col_f[:], 1.0)
            ones_col = const.tile([128, 1], F32R)
            nc.vector.tensor_copy(ones_col[:], ones_col_f[:])
            ones_row = const.tile([1, 128], F32)
            nc.vector.memset(ones_row[:], 1.0)
            eps_col = const.tile([128, 1], F32)
            nc.vector.memset(eps_col[:], EPS)
            negcap_col = const.tile([128, 1], F32)
            nc.vector.memset(negcap_col[:], -MAX_ATTN)

            # ---------------- S0: hn = rms_norm(x) (pre-attn scale folded into
            # wq/wk/wv on host), transposed into hnT ----------------
            hnT_cm = tc.tile_pool(name="hnT_pool", bufs=1)
            hnT_pool = hnT_cm.__enter__()
            hnT = hnT_pool.tile([128, DC, T], F32R, name="hnT")
            with (
                tc.tile_pool(name="s0", bufs=2) as s0,
                tc.tile_pool(name="s0b", bufs=1) as s0b,
                tc.tile_pool(name="ps0", bufs=4, space="PSUM") as ps0,
            ):
                for t in range(TC):
                    x_t = s0.tile([128, D], F32, tag="x_t")
                    nc.sync.dma_start(x_t[:], x_d[t * 128:(t + 1) * 128, :])
                    hn_t = s0b.tile([128, D], F32, tag="hn_t")
                    ms_t = msp.tile([128, 1], F32, tag="ms")
                    # hn_t <- x^2 (scratch), ms_t <- rowsum(x^2)
                    nc.vector.scalar_tensor_tensor(
                        hn_t[:], x_t[:], 1.0, x_t[:],
                        op0=ALU.mult, op1=ALU.mult, accum_out=ms_t[:])
                    inv_t = msp.tile([128, 1], F32, tag="inv")
                    nc.scalar.activation(inv_t[:], ms_t[:], AF.Sqrt,
                                         bias=eps_col[:], scale=1.0 / D)
                    nc.vector.reciprocal(inv_t[:], inv_t[:])
                    nc.vector.tensor_scalar_mul(hn_t[:], x_t[:], inv_t[:])
                    for dc in range(DC):
                        pt = ps0.tile([128, 128], F32, tag="pt")
                        nc.tensor.transpose(
                            pt[:], hn_t[:, dc * 128:(dc + 1) * 128], ident[:])
                        nc.vector.tensor_copy(
                            hnT[:, dc, t * 128:(t + 1) * 128], pt[:])

            # ---------------- S1: QT/KT (transposed + rope) and V ------------
            with (
                tc.tile_pool(name="s1t", bufs=3) as s1t,
                tc.tile_pool(name="s1c", bufs=1) as s1c,
                tc.tile_pool(name="ps1", bufs=2, space="PSUM") as ps1,
            ):
                s1w_cm = tc.tile_pool(name="s1w", bufs=2)
                s1w = s1w_cm.__enter__()
                cosT_sb = s1c.tile([HD, T], F32)
                nc.sync.dma_start(cosT_sb[:], cosT_d[:])
                sinTn_sb = s1c.tile([HD, T], F32)
                nc.sync.dma_start(sinTn_sb[:], sinTn_d[:])

                def rope_store(psum, dst, g):
                    """psum [128 d, 512 tok] -> rope -> DMA to dst [128, 512]."""
                    cs = cosT_sb[:, g * 512:(g + 1) * 512]
                    sn = sinTn_sb[:, g * 512:(g + 1) * 512]
                    raw = s1t.tile([128, 512], F32, tag="rp_raw")
                    nc.vector.tensor_copy(raw[:], psum[:])
                    rot = s1t.tile([128, 512], F32, tag="rp_rot")
                    nc.sync.dma_start(rot[0:64, :], raw[64:128, :])
                    nc.sync.dma_start(rot[64:128, :], raw[0:64, :])
                    oz = s1t.tile([128, 512], F32R, tag="rp_out")
                    nc.vector.tensor_tensor(oz[:], raw[:], cs, ALU.mult)
                    nc.vector.tensor_tensor(rot[:], rot[:], sn, ALU.mult)
                    nc.vector.tensor_tensor(oz[:], oz[:], rot[:], ALU.add)
                    nc.sync.dma_start(dst, oz[:])

                for cb in range(HQ + HKV):  # q heads then kv heads
                    is_q = cb < HQ
                    w_d = wq_d if is_q else wk_d
                    col = cb * 128 if is_q else (cb - HQ) * 128
                    w_cb = s1w.tile([128, DC, 128], F32R, tag="w_cb")
                    nc.sync.dma_start(
                        w_cb[:],
                        w_d[:, col:col + 128].rearrange(
                            "(dc p) c -> p dc c", p=128))
                    for g in range(TG):
                        pq = ps1.tile([128, 512], F32, tag="pq")
                        for dc in range(DC):
                            nc.tensor.matmul(
                                pq[:], w_cb[:, dc, :],
                                hnT[:, dc, g * 512:(g + 1) * 512],
                                start=(dc == 0), stop=(dc == DC - 1))
                        dst = (qt_dram[cb, :, g * 512:(g + 1) * 512] if is_q
                               else kt_dram[cb - HQ, :, g * 512:(g + 1) * 512])
                        rope_store(pq, dst, g)

                s1w_cm.__exit__(None, None, None)
                s1v_cm = tc.tile_pool(name="s1v", bufs=1)
                s1v = s1v_cm.__enter__()
                # V token-major
                wv_sb = s1v.tile([128, DC, HKV * HD], F32R)
                nc.sync.dma_start(
                    wv_sb[:], wv_d.rearrange("(dc p) c -> p dc c", p=128))
                for t in range(TC):
                    pv = ps1.tile([128, 256], F32, tag="pv")
                    for dc in range(DC):
                        nc.tensor.matmul(
                            pv[:], hnT[:, dc, t * 128:(t + 1) * 128],
                            wv_sb[:, dc, :],
                            start=(dc == 0), stop=(dc == DC - 1))
                    vz = s1t.tile([128, 256], F32R, tag="vz")
                    nc.vector.tensor_copy(vz[:], pv[:])
                    nc.sync.dma_start(v_dram[t * 128:(t + 1) * 128, :], vz[:])
                s1v_cm.__exit__(None, None, None)

            # ---------------- S2: attention (transposed softmax) -------------
            hnT_cm.__exit__(None, None, None)
            attnT_cm = tc.tile_pool(name="attnT_pool", bufs=1)
            attnT_pool = attnT_cm.__enter__()
            attnT = attnT_pool.tile([128, HQ, T], F32R, name="attnT")
            with (
                tc.tile_pool(name="s2c", bufs=1) as s2c,
                tc.tile_pool(name="s2t", bufs=3) as s2t,
                tc.tile_pool(name="ps2s", bufs=2, space="PSUM") as ps2s,
                tc.tile_pool(name="ps2o", bufs=2, space="PSUM") as ps2o,
                tc.tile_pool(name="ps2m", bufs=2, space="PSUM") as ps2m,
            ):
                kt_sb = s2c.tile([128, HKV, T], F32R)
                v_sb = s2c.tile([128, KC, HKV * HD], F32R)
                nc.sync.dma_start(
                    kt_sb[:], kt_dram[:].rearrange("h p t -> p h t"))
                nc.sync.dma_start(
                    v_sb[:], v_dram[:].rearrange("(kc p) c -> p kc c", p=128))
                dmask_sb = s2c.tile([128, n_mask_tiles, 512], F32)
                nc.sync.dma_start(
                    dmask_sb[:], dmask_d[:].rearrange("n p q -> p n q"))

                for h in range(HQ):
                    kv = h // GQ
                    for g in range(TG):
                        ks = [k for k in range(KC)
                              if blocks[(g, k)][0] != "skip"]
                        qt_sb = s2t.tile([128, 512], F32R, tag="qt")
                        nc.sync.dma_start(
                            qt_sb[:], qt_dram[h, :, g * 512:(g + 1) * 512])
                        po = ps2o.tile([128, 512], F32, tag="po")
                        psum = ps2m.tile([1, 512], F32, tag="psums")
                        for i, k in enumerate(ks):
                            kind, mi = blocks[(g, k)]
                            psc = ps2s.tile([128, 512], F32, tag="psc")
                            nc.tensor.matmul(
                                psc[:],
                                kt_sb[:, kv, k * 128:(k + 1) * 128],
                                qt_sb[:], start=True, stop=True)
                            sc_t = s2t.tile([128, 512], F32, tag="sc")
                            nc.scalar.activation(sc_t[:], psc[:], AF.Tanh,
                                                 scale=ATTN_MULT / MAX_ATTN)
                            p_t = s2t.tile([128, 512], F32R, tag="pt")
                            nc.scalar.activation(p_t[:], sc_t[:], AF.Exp,
                                                 scale=MAX_ATTN,
                                                 bias=negcap_col[:])
                            if kind == "partial":
                                nc.vector.tensor_tensor(
                                    p_t[:], p_t[:], dmask_sb[:, mi, :],
                                    ALU.mult)
                            first = i == 0
                            last = i == len(ks) - 1
                            nc.tensor.matmul(
                                po[:],
                                v_sb[:, k, kv * 128:(kv + 1) * 128],
                                p_t[:], start=first, stop=last)
                            nc.tensor.matmul(
                                psum[:], ones_col[:], p_t[:],
                                start=first, stop=last)
                        recip = s2t.tile([1, 512], F32, tag="recip")
                        nc.vector.reciprocal(recip[:], psum[:])
                        pbc = ps2s.tile([128, 512], F32, tag="pbc")
                        nc.tensor.matmul(pbc[:], ones_row[:], recip[:],
                                         start=True, stop=True)
                        rb = s2t.tile([128, 512], F32, tag="rb")
                        nc.vector.tensor_copy(rb[:], pbc[:])
                        nc.vector.tensor_tensor(
                            attnT[:, h, g * 512:(g + 1) * 512], po[:], rb[:],
                            ALU.mult)

            # ---------------- S3: wo (row shard) -> partial h_attn -> AR1 ----
            with (
                tc.tile_pool(name="s3w", bufs=2) as s3w,
                tc.tile_pool(name="s3t", bufs=3) as s3t,
                tc.tile_pool(name="ps3", bufs=2, space="PSUM") as ps3,
            ):
                for db in range(DB):
                    wo_db = s3w.tile([128, HQ, 512], F32R, tag="wo_db")
                    nc.sync.dma_start(
                        wo_db[:],
                        wo_d[:, db * 512:(db + 1) * 512].rearrange(
                            "(h p) d -> p h d", p=128))
                    for t in range(TC):
                        pw = ps3.tile([128, 512], F32, tag="pw")
                        for h in range(HQ):
                            nc.tensor.matmul(
                                pw[:], attnT[:, h, t * 128:(t + 1) * 128],
                                wo_db[:, h, :],
                                start=(h == 0), stop=(h == HQ - 1))
                        oz = s3t.tile([128, 512], F32, tag="oz")
                        nc.vector.tensor_copy(oz[:], pw[:])
                        nc.sync.dma_start(
                            ar1_in[t * 128:(t + 1) * 128,
                                   db * 512:(db + 1) * 512], oz[:])
            nc.gpsimd.collective_compute(
                "AllReduce", ALU.add, replica_groups=RG,
                ins=[ar1_in[:].opt()], outs=[ar1_out[:].opt()])

            # ---------------- S4: h1 = x + rms(ha)*s_post; hn2T (bf16) -------
            attnT_cm.__exit__(None, None, None)
            hn2T_cm = tc.tile_pool(name="hn2T_pool", bufs=1)
            hn2T_pool = hn2T_cm.__enter__()
            hn2T = hn2T_pool.tile([128, DC, T], BF16, name="hn2T")
            with (
                tc.tile_pool(name="s4", bufs=2) as s4,
                tc.tile_pool(name="s4b", bufs=1) as s4b,
                tc.tile_pool(name="ps4", bufs=4, space="PSUM") as ps4,
            ):
                spa_sb = s4b.tile([128, D], F32)
                nc.sync.dma_start(spa_sb[:], spa_d[:])
                for t in range(TC):
                    ha_t = s4b.tile([128, D], F32, tag="ha_t")
                    nc.sync.dma_start(
                        ha_t[:], ar1_out[t * 128:(t + 1) * 128, :])
                    x_t = s4.tile([128, D], F32, tag="x_t")
                    nc.sync.dma_start(x_t[:], x_d[t * 128:(t + 1) * 128, :])
                    scr = s4b.tile([128, D], F32, tag="scr")
                    ms_t = msp.tile([128, 1], F32, tag="ms4")
                    nc.vector.scalar_tensor_tensor(
                        scr[:], ha_t[:], 1.0, ha_t[:],
                        op0=ALU.mult, op1=ALU.mult, accum_out=ms_t[:])
                    inv_t = msp.tile([128, 1], F32, tag="inv4")
                    nc.scalar.activation(inv_t[:], ms_t[:], AF.Sqrt,
                                         bias=eps_col[:], scale=1.0 / D)
                    nc.vector.reciprocal(inv_t[:], inv_t[:])
                    h1_t = s4b.tile([128, D], F32, tag="h1_t")
                    nc.vector.scalar_tensor_tensor(
                        h1_t[:], ha_t[:], inv_t[:], spa_sb[:],
                        op0=ALU.mult, op1=ALU.mult)
                    nc.vector.tensor_tensor(h1_t[:], h1_t[:], x_t[:], ALU.add)
                    nc.sync.dma_start(
                        h1_dram[t * 128:(t + 1) * 128, :], h1_t[:])
                    ms2_t = msp.tile([128, 1], F32, tag="ms4b")
                    nc.vector.scalar_tensor_tensor(
                        scr[:], h1_t[:], 1.0, h1_t[:],
                        op0=ALU.mult, op1=ALU.mult, accum_out=ms2_t[:])
                    inv2_t = msp.tile([128, 1], F32, tag="inv4b")
                    nc.scalar.activation(inv2_t[:], ms2_t[:], AF.Sqrt,
                                         bias=eps_col[:], scale=1.0 / D)
                    nc.vector.reciprocal(inv2_t[:], inv2_t[:])
                    hn2b_t = s4.tile([128, D], BF16, tag="hn2b")
                    nc.vector.tensor_scalar_mul(hn2b_t[:], h1_t[:], inv2_t[:])
                    for dc in range(DC):
                        ptb = ps4.tile([128, 128], BF16, tag="ptb")
                        nc.tensor.transpose(
                            ptb[:], hn2b_t[:, dc * 128:(dc + 1) * 128],
                            ident_bf[:])
                        nc.vector.tensor_copy(
                            hn2T[:, dc, t * 128:(t + 1) * 128], ptb[:])

            # ---------------- S5: gateT/valT -> gelu*val -> actT (bf16) ------
            actT_cm = tc.tile_pool(name="actT_pool", bufs=1)
            actT_pool = actT_cm.__enter__()
            actT = actT_pool.tile([128, FC, T], BF16, name="actT")
            with (
                tc.tile_pool(name="s5w", bufs=2) as s5w,
                tc.tile_pool(name="s5t", bufs=2) as s5t,
                tc.tile_pool(name="ps5", bufs=4, space="PSUM") as ps5,
            ):
                for f in range(FC):
                    wg_f = s5w.tile([128, DC, 128], BF16, tag="wg_f")
                    nc.sync.dma_start(
                        wg_f[:], wg_d[:, f * 128:(f + 1) * 128].rearrange(
                            "(dc p) c -> p dc c", p=128))
                    wv2_f = s5w.tile([128, DC, 128], BF16, tag="wv2_f")
                    nc.sync.dma_start(
                        wv2_f[:], wv2_d[:, f * 128:(f + 1) * 128].rearrange(
                            "(dc p) c -> p dc c", p=128))
                    for g in range(TG):
                        pg = ps5.tile([128, 512], F32, tag="pg")
                        pv2 = ps5.tile([128, 512], F32, tag="pv2")
                        for dc in range(DC):
                            nc.tensor.matmul(
                                pg[:], wg_f[:, dc, :],
                                hn2T[:, dc, g * 512:(g + 1) * 512],
                                start=(dc == 0), stop=(dc == DC - 1))
                        for dc in range(DC):
                            nc.tensor.matmul(
                                pv2[:], wv2_f[:, dc, :],
                                hn2T[:, dc, g * 512:(g + 1) * 512],
                                start=(dc == 0), stop=(dc == DC - 1))
                        gel = s5t.tile([128, 512], F32, tag="gel")
                        nc.scalar.activation(gel[:], pg[:], AF.Gelu_apprx_tanh)
                        nc.vector.tensor_tensor(
                            actT[:, f, g * 512:(g + 1) * 512], gel[:], pv2[:],
                            ALU.mult)

            # ---------------- S6: w_out (row shard) -> partial h_dense -> AR2
            with (
                tc.tile_pool(name="s6w", bufs=2) as s6w,
                tc.tile_pool(name="s6t", bufs=3) as s6t,
                tc.tile_pool(name="ps6", bufs=2, space="PSUM") as ps6,
            ):
                for db in range(DB):
                    wout_db = s6w.tile([128, FC, 512], BF16, tag="wout_db")
                    nc.sync.dma_start(
                        wout_db[:],
                        wout_d[:, db * 512:(db + 1) * 512].rearrange(
                            "(f p) d -> p f d", p=128))
                    for t in range(TC):
                        pd = ps6.tile([128, 512], F32, tag="pd")
                        for f in range(FC):
                            nc.tensor.matmul(
                                pd[:], actT[:, f, t * 128:(t + 1) * 128],
                                wout_db[:, f, :],
                                start=(f == 0), stop=(f == FC - 1))
                        oz = s6t.tile([128, 512], F32, tag="oz6")
                        nc.vector.tensor_copy(oz[:], pd[:])
                        nc.sync.dma_start(
                            ar2_in[t * 128:(t + 1) * 128,
                                   db * 512:(db + 1) * 512], oz[:])
            nc.gpsimd.collective_compute(
                "AllReduce", ALU.add, replica_groups=RG,
                ins=[ar2_in[:].opt()], outs=[ar2_out[:].opt()])

            actT_cm.__exit__(None, None, None)
            hn2T_cm.__exit__(None, None, None)
            # ---------------- S7: out = h1 + rms(h_dense)*s_post_mlp ---------
            with (
                tc.tile_pool(name="s7", bufs=2) as s7,
                tc.tile_pool(name="s7b", bufs=1) as s7b,
            ):
                spm_sb = s7b.tile([128, D], F32)
                nc.sync.dma_start(spm_sb[:], spm_d[:])
                for t in range(TC):
                    hd_t = s7.tile([128, D], F32, tag="hd_t")
                    nc.sync.dma_start(
                        hd_t[:], ar2_out[t * 128:(t + 1) * 128, :])
                    h1_t = s7.tile([128, D], F32, tag="h1r")
                    nc.sync.dma_start(
                        h1_t[:], h1_dram[t * 128:(t + 1) * 128, :])
                    scr = s7b.tile([128, D], F32, tag="scr7")
                    ms_t = msp.tile([128, 1], F32, tag="ms7")
                    nc.vector.scalar_tensor_tensor(
                        scr[:], hd_t[:], 1.0, hd_t[:],
                        op0=ALU.mult, op1=ALU.mult, accum_out=ms_t[:])
                    inv_t = msp.tile([128, 1], F32, tag="inv7")
                    nc.scalar.activation(inv_t[:], ms_t[:], AF.Sqrt,
                                         bias=eps_col[:], scale=1.0 / D)
                    nc.vector.reciprocal(inv_t[:], inv_t[:])
                    o_t = s7.tile([128, D], F32, tag="o_t")
                    nc.vector.scalar_tensor_tensor(
                        o_t[:], hd_t[:], inv_t[:], spm_sb[:],
                        op0=ALU.mult, op1=ALU.mult)
                    nc.vector.tensor_tensor(o_t[:], o_t[:], h1_t[:], ALU.add)
                    nc.sync.dma_start(out_d[t * 128:(t + 1) * 128, :], o_t[:])

    nc.compile()
    return nc


_NC_CACHE = {}
_FN_CACHE = {}
LAST_RESULTS = None


def _get_sharded_fn(nc):
    """Build (once) the jitted shard_map callable for `nc` across 8 cores.

    Mirrors concourse.bass2jax.run_bass_via_pjrt's multi-core path, but caches
    the compiled function and takes pre-sharded device arrays so repeated calls
    can be timed without re-shipping inputs.
    """
    if id(nc) in _FN_CACHE:
        return _FN_CACHE[id(nc)]
    import jax
    from jax.sharding import Mesh, PartitionSpec
    from jax.experimental.shard_map import shard_map
    from concourse import bass2jax as b2j

    b2j.install_neuronx_cc_hook()
    part_name = nc.partition_id_tensor.name if nc.partition_id_tensor else None
    in_names, out_names, out_avals, zero_outs = [], [], [], []
    for alloc in nc.m.functions[0].allocations:
        if not isinstance(alloc, mybir.MemoryLocationSet):
            continue
        name = alloc.memorylocations[0].name
        if alloc.kind == "ExternalInput":
            if name == part_name:
                continue
            in_names.append(name)
        elif alloc.kind == "ExternalOutput":
            out_names.append(name)
            shape = tuple(alloc.tensor_shape)
            dtype = mybir.dt.np(alloc.dtype)
            out_avals.append(jax.core.ShapedArray(shape, dtype))
            zero_outs.append(np.zeros(shape, dtype))
    n_params = len(in_names)
    all_names = in_names + out_names
    if part_name is not None:
        all_names = all_names + [part_name]

    def _body(*args):
        operands = list(args)
        if part_name is not None:
            operands.append(b2j.partition_id_tensor())
        outs = b2j._bass_exec_p.bind(
            *operands,
            out_avals=tuple(out_avals),
            in_names=tuple(all_names),
            out_names=tuple(out_names),
            lowering_input_output_aliases=(),
            sim_require_finite=True,
            sim_require_nnan=True,
            nc=nc,
        )
        return tuple(outs)

    devices = jax.devices()[:N_CORES]
    mesh = Mesh(np.asarray(devices), ("core",))
    n_outs = len(out_names)
    donate = tuple(range(n_params, n_params + n_outs))
    sharded = jax.jit(
        shard_map(
            _body,
            mesh=mesh,
            in_specs=(PartitionSpec("core"),) * (n_params + n_outs),
            out_specs=(PartitionSpec("core"),) * n_outs,
            check_rep=False,
        ),
        donate_argnums=donate,
        keep_unused=True,
    )
    entry = dict(
        fn=sharded, in_names=in_names, out_names=out_names,
        out_avals=out_avals, zero_outs=zero_outs, mesh=mesh,
    )
    _FN_CACHE[id(nc)] = entry
    return entry


def _device_inputs(nc, in_maps):
    import jax
    from jax.sharding import NamedSharding, PartitionSpec

    entry = _get_sharded_fn(nc)
    sh = NamedSharding(entry["mesh"], PartitionSpec("core"))
    concat_in = [
        np.concatenate([np.asarray(m[name]) for m in in_maps], axis=0)
        for name in entry["in_names"]
    ]
    return [jax.device_put(a, sh) for a in concat_in]


def _dev_zeros(nc):
    import jax
    from jax.sharding import NamedSharding, PartitionSpec

    entry = _get_sharded_fn(nc)
    sh = NamedSharding(entry["mesh"], PartitionSpec("core"))
    return [
        jax.device_put(
            np.zeros((N_CORES * z.shape[0], *z.shape[1:]), z.dtype), sh)
        for z in entry["zero_outs"]
    ]


def _run(nc, dev_in):
    entry = _get_sharded_fn(nc)
    out_arrs = entry["fn"](*dev_in, *_dev_zeros(nc))
    outs = []
    for i, name in enumerate(entry["out_names"]):
        shp = entry["out_avals"][i].shape
        outs.append(np.asarray(out_arrs[i]).reshape(N_CORES, *shp))
    return dict(zip(entry["out_names"], outs))


def _run_timed(nc, dev_in, iters=5):
    """Returns (per-call wall seconds list). Inputs already device-resident;
    donated zero buffers are re-staged outside the timed window."""
    import time as _time

    entry = _get_sharded_fn(nc)
    times = []
    for _ in range(iters):
        zeros = _dev_zeros(nc)
        for z in zeros:
            z.block_until_ready()
        t0 = _time.perf_counter()
        out = entry["fn"](*dev_in, *zeros)
        for o in out:
            o.block_until_ready()
        times.append(_time.perf_counter() - t0)
    return times


def _run_timed_pipelined(nc, dev_in, iters=8):
    """Enqueue `iters` executions back-to-back (async dispatch), block once.
    Returns (total_s, per_iter_slope_s) where slope excludes one-time overhead:
    slope = (t_N - t_1) / (N - 1)."""
    import time as _time

    entry = _get_sharded_fn(nc)
    zsets = [_dev_zeros(nc) for _ in range(iters)]
    for zs in zsets:
        for z in zs:
            z.block_until_ready()
    # one warm call
    out = entry["fn"](*dev_in, *_dev_zeros(nc))
    for o in out:
        o.block_until_ready()

    t0 = _time.perf_counter()
    out = entry["fn"](*dev_in, *zsets[0])
    for o in out:
        o.block_until_ready()
    t1 = _time.perf_counter()

    outs = []
    for i in range(1, iters):
        outs.append(entry["fn"](*dev_in, *zsets[i]))
    for os_ in outs:
        for o in os_:
            o.block_until_ready()
    t2 = _time.perf_counter()
    one = t1 - t0
    slope = (t2 - t1) / (iters - 1) if iters > 1 else one
    return one, slope


def _prepare(inputs):
    x = np.asarray(inputs["x"], np.float32)
    mask_qk = np.asarray(inputs["mask"]).reshape(S, S).astype(bool)
    s_pre_attn = np.asarray(inputs["scale_pre_attn"], np.float32)
    s_post_attn = np.asarray(inputs["scale_post_attn"], np.float32)
    s_pre_mlp = np.asarray(inputs["scale_pre_mlp"], np.float32)
    s_post_mlp = np.asarray(inputs["scale_post_mlp"], np.float32)
    wq = np.asarray(inputs["wq"], np.float32) * s_pre_attn[:, None]
    wk = np.asarray(inputs["wk"], np.float32) * s_pre_attn[:, None]
    wv = np.asarray(inputs["wv"], np.float32) * s_pre_attn[:, None]
    wo = np.asarray(inputs["wo"], np.float32)
    wg = np.asarray(inputs["w_gate"], np.float32) * s_pre_mlp[:, None]
    wv2 = np.asarray(inputs["w_val"], np.float32) * s_pre_mlp[:, None]
    wout = np.asarray(inputs["w_out"], np.float32)

    blocks, dmask = _analyze_mask(mask_qk)
    key = tuple(sorted((k, v[0], v[1]) for k, v in blocks.items()))
    if key not in _NC_CACHE:
        _NC_CACHE[key] = _build_nc(blocks, dmask.shape[0])
    nc = _NC_CACHE[key]

    # FFN zero-padding to a multiple of 512 (22*128 per TP rank)
    wg_p = np.zeros((D, FFN_PAD), ml_dtypes.bfloat16)
    wg_p[:, :FFN] = wg.astype(ml_dtypes.bfloat16)
    wv2_p = np.zeros((D, FFN_PAD), ml_dtypes.bfloat16)
    wv2_p[:, :FFN] = wv2.astype(ml_dtypes.bfloat16)
    wout_p = np.zeros((FFN_PAD, D), ml_dtypes.bfloat16)
    wout_p[:FFN, :] = wout.astype(ml_dtypes.bfloat16)

    # RoPE tables in T-layout
    inv_freq = 1.0 / (BASE ** (np.arange(0, HD, 2, dtype=np.float64) / HD))
    phase = np.arange(S, dtype=np.float64)[:, None] * inv_freq[None, :]
    cos_f = np.cos(phase).astype(np.float32)   # [S, 64]
    sin_f = np.sin(phase).astype(np.float32)
    cosT = np.concatenate([cos_f.T, cos_f.T], axis=0)           # [128, S]
    sinTn = np.concatenate([-sin_f.T, sin_f.T], axis=0)         # [128, S]

    spa_bc = np.ascontiguousarray(
        np.broadcast_to(s_post_attn, (128, D)), dtype=np.float32)
    spm_bc = np.ascontiguousarray(
        np.broadcast_to(s_post_mlp, (128, D)), dtype=np.float32)

    in_maps = []
    for c in range(N_CORES):
        b, m = c // TP, c % TP
        in_maps.append({
            "x": np.ascontiguousarray(x[b]),
            "wq": np.ascontiguousarray(wq[:, m * HQ * HD:(m + 1) * HQ * HD]),
            "wk": np.ascontiguousarray(wk[:, m * HKV * HD:(m + 1) * HKV * HD]),
            "wv": np.ascontiguousarray(wv[:, m * HKV * HD:(m + 1) * HKV * HD]),
            "wo": np.ascontiguousarray(wo[m * HQ * HD:(m + 1) * HQ * HD, :]),
            "wg": np.ascontiguousarray(wg_p[:, m * F:(m + 1) * F]),
            "wv2": np.ascontiguousarray(wv2_p[:, m * F:(m + 1) * F]),
            "wout": np.ascontiguousarray(wout_p[m * F:(m + 1) * F, :]),
            "sp_attn": spa_bc,
            "sp_mlp": spm_bc,
            "cosT": cosT,
            "sinTn": sinTn,
            "dmask": dmask,
        })

    return nc, in_maps


def kernel(**inputs):
    global LAST_RESULTS
    nc, in_maps = _prepare(inputs)
    dev_in = _device_inputs(nc, in_maps)
    res = _run(nc, dev_in)
    LAST_RESULTS = res
    out = np.stack([res["out"][0], res["out"][TP]])
    return out.astype(np.float32)


# revision 22
# speedup vs baseline: 45.9787x; 45.9787x over previous
"""Trainium2 Bass kernel for a dense decoder layer (GQA attention + gated MLP).

Distribution (8 NeuronCores): DP=2 over batch x TP=4 megatron split.
  - cores 0-3: batch 0, model ranks 0-3; cores 4-7: batch 1.
  - wq/wk/wv column-sharded by heads (8 q / 2 kv heads per core), wo row-sharded;
    w_gate/w_val column-sharded, w_out row-sharded (FFN padded 10928->11264, 2816/core).
  - Two AllReduces (partial h_attn, partial h_dense) within each 4-core group.

On-device dataflow (per core, its full batch of 1024 tokens):
  S0  x -> rms_norm -> hn, PE-transposed to hnT [D, T] (fp32, SBUF resident)
  S1  QT/KT produced directly transposed (lhsT=W chunk, rhs=hnT), RoPE applied in
      T-layout (partition-shift DMA + DVE), V token-major; spilled to DRAM
  S2  attention per head, transposed-softmax: scoresT=[kv,q] via matmul, the 30*tanh
      soft-cap bounds logits so exp(l-30) needs no max pass; row sums via ones-matmul;
      causal handled by block skip + 0/1 mask tiles on partial blocks
  S3  wo row-shard matmul -> partial h_attn -> AllReduce#1
  S4  h1 = x + rms(h_attn)*s_post_attn; hn2 (pre-mlp scale folded into weights) cast
      bf16, PE-transposed to hn2T
  S5  gateT/valT (bf16 matmuls, fp32 psum), gelu_tanh * val -> actT bf16 (SBUF)
  S6  w_out row-shard matmul -> partial h_dense -> AllReduce#2
  S7  out = h1 + rms(h_dense)*s_post_mlp

All big matmuls use float32r (full PE rate, ~1e-3 worst-case rel err); the MLP runs
bf16 weights/activations with fp32 accumulation.
"""

import os

import numpy as np
import ml_dtypes

import concourse.bass as bass
import concourse.mybir as mybir
import concourse.tile as tile
from concourse import bacc
from concourse.bass_utils import run_bass_kernel_spmd
from concourse.masks import make_identity

F32 = mybir.dt.float32
F32R = mybir.dt.float32r
BF16 = mybir.dt.bfloat16
AF = mybir.ActivationFunctionType
ALU = mybir.AluOpType

# Problem dims
B, S, D = 2, 1024, 4096
NQ, NKV, HD = 32, 8, 128
FFN = 10928
ATTN_MULT = 0.08838834764831845
MAX_ATTN = 30.0
EPS = 1e-5
BASE = 10000.0

# Distribution
N_CORES = 8
TP = 4
RG = [[0, 1, 2, 3], [4, 5, 6, 7]]

# Per-core dims
T = S                      # tokens per core (its whole batch)
TC = T // 128              # 8 token chunks
TG = T // 512              # 2 token groups (matmul moving dim)
DC = D // 128              # 32 contraction chunks
DB = D // 512              # 8 output blocks
HQ = NQ // TP              # 8 q heads per core
HKV = NKV // TP            # 2 kv heads per core
GQ = NQ // NKV             # 4 q heads per kv head
FFN_PAD = 11264            # 22*128*4
F = FFN_PAD // TP          # 2816 ffn columns per core
FC = F // 128              # 22 ffn chunks
KC = S // 128              # 8 kv chunks



def _dma_split(nc, dst, src, n=4):
    """Issue n parallel dma_starts covering dst/src sliced on their 2nd axis."""
    dims = dst.shape
    ax = 1
    size = dims[ax]
    step = max(1, size // n)
    i = 0
    while i < size:
        j = min(size, i + step)
        if len(dims) == 2:
            nc.sync.dma_start(dst[:, i:j], src[:, i:j])
        else:
            nc.sync.dma_start(dst[:, i:j, :], src[:, i:j, :])
        i = j


def _analyze_mask(mask_qk: np.ndarray):
    """Classify each (q-group-of-512, kv-chunk-of-128) block of mask[q, kv]:
    'skip' (all masked), 'full' (all visible), or a [128, 512] 0/1 tile in
    scoresT layout [kv, q]."""
    blocks = {}
    tiles = []
    for g in range(TG):
        for k in range(KC):
            blk = mask_qk[g * 512:(g + 1) * 512, k * 128:(k + 1) * 128]
            if not blk.any():
                blocks[(g, k)] = ("skip", -1)
            elif blk.all():
                blocks[(g, k)] = ("full", -1)
            else:
                blocks[(g, k)] = ("partial", len(tiles))
                tiles.append(blk.T.astype(np.float32))  # [kv 128, q 512]
    if not tiles:
        tiles.append(np.zeros((128, 512), np.float32))
    return blocks, np.stack(tiles)


def _build_nc(blocks, n_mask_tiles, sim_no_ar=False):
    nc = bacc.Bacc("TRN2", target_bir_lowering=False, debug=False,
                   num_devices=N_CORES)

    # Per-core external inputs
    x_d = nc.dram_tensor("x", [T, D], F32, kind="ExternalInput")
    wq_d = nc.dram_tensor("wq", [HQ, 128, DC, 128], F32R, kind="ExternalInput")
    wk_d = nc.dram_tensor("wk", [HKV, 128, DC, 128], F32R, kind="ExternalInput")
    wv_d = nc.dram_tensor("wv", [128, DC, HKV * HD], F32R, kind="ExternalInput")
    wo_d = nc.dram_tensor("wo", [DB, 128, HQ, 512], F32R, kind="ExternalInput")
    wg_d = nc.dram_tensor("wg", [FC, 128, DC, 128], BF16, kind="ExternalInput")
    wv2_d = nc.dram_tensor("wv2", [FC, 128, DC, 128], BF16, kind="ExternalInput")
    wout_d = nc.dram_tensor("wout", [DB, 128, FC, 512], BF16, kind="ExternalInput")
    spa_d = nc.dram_tensor("sp_attn", [128, D], F32, kind="ExternalInput")
    spm_d = nc.dram_tensor("sp_mlp", [128, D], F32, kind="ExternalInput")
    cosT_d = nc.dram_tensor("cosT", [HD, T], F32, kind="ExternalInput")
    sinTn_d = nc.dram_tensor("sinTn", [HD, T], F32, kind="ExternalInput")
    dmask_d = nc.dram_tensor("dmask", [n_mask_tiles, 128, 512], F32,
                             kind="ExternalInput")
    out_d = nc.dram_tensor("out", [T, D], F32, kind="ExternalOutput")


    with tile.TileContext(nc) as tc:
        with (
            tc.tile_pool(name="dram", bufs=1, space="DRAM") as dram,
            tc.tile_pool(name="const", bufs=1) as const,
            tc.tile_pool(name="ms", bufs=4) as msp,
        ):
            # DRAM scratch
            qt_dram = dram.tile([HQ, HD, T], F32R)
            kt_dram = dram.tile([HKV, HD, T], F32R)
            v_dram = dram.tile([T, HKV * HD], F32R)
            DH = D // 2
            ar1_in = [dram.tile([T, DH], F32, name=f"ar1i{h}") for h in range(2)]
            ar1_out = [dram.tile([T, DH], F32, name=f"ar1o{h}") for h in range(2)]
            ar2_in = [dram.tile([T, DH], F32, name=f"ar2i{h}") for h in range(2)]
            ar2_out = [dram.tile([T, DH], F32, name=f"ar2o{h}") for h in range(2)]
            h1_dram = dram.tile([T, D], F32)
            if sim_no_ar:
                ar1_out = ar1_in
                ar2_out = ar2_in

            def _ar(in_t, out_t):
                if not sim_no_ar:
                    nc.gpsimd.collective_compute(
                        "AllReduce", ALU.add, replica_groups=RG,
                        ins=[in_t[:].opt()], outs=[out_t[:].opt()])

            ident = const.tile([128, 128], F32)
            make_identity(nc, ident)
            ident_bf = const.tile([128, 128], BF16)
            nc.vector.tensor_copy(ident_bf[:], ident[:])
            ones_col_f = const.tile([128, 1], F32)
            nc.vector.memset(ones_col_f[:], 1.0)
            ones_col = const.tile([128, 1], F32R)
            nc.vector.tensor_copy(ones_col[:], ones_col_f[:])
            ones_row = const.tile([1, 128], F32)
            nc.vector.memset(ones_row[:], 1.0)
            eps_col = const.tile([128, 1], F32)
            nc.vector.memset(eps_col[:], EPS)
            negcap_col = const.tile([128, 1], F32)
            nc.vector.memset(negcap_col[:], -MAX_ATTN)

            # ---------------- S0: hn = rms_norm(x) (pre-attn scale folded into
            # wq/wk/wv on host), transposed into hnT ----------------
            hnT_cm = tc.tile_pool(name="hnT_pool", bufs=1)
            hnT_pool = hnT_cm.__enter__()
            hnT = hnT_pool.tile([128, DC, T], F32R, name="hnT")
            with (
                tc.tile_pool(name="s0", bufs=2) as s0,
                tc.tile_pool(name="s0b", bufs=2) as s0b,
                tc.tile_pool(name="ps0", bufs=4, space="PSUM") as ps0,
            ):
                for t in range(TC):
                    x_t = s0.tile([128, D], F32, tag="x_t")
                    _dma_split(nc, x_t[:], x_d[t * 128:(t + 1) * 128, :], 4)
                    hn_t = s0b.tile([128, D], F32, tag="hn_t")
                    ms_t = msp.tile([128, 1], F32, tag="ms")
                    # hn_t <- x^2 (scratch), ms_t <- rowsum(x^2)
                    nc.vector.scalar_tensor_tensor(
                        hn_t[:], x_t[:], 1.0, x_t[:],
                        op0=ALU.mult, op1=ALU.mult, accum_out=ms_t[:])
                    inv_t = msp.tile([128, 1], F32, tag="inv")
                    nc.scalar.activation(inv_t[:], ms_t[:], AF.Sqrt,
                                         bias=eps_col[:], scale=1.0 / D)
                    nc.vector.reciprocal(inv_t[:], inv_t[:])
                    nc.vector.tensor_scalar_mul(hn_t[:], x_t[:], inv_t[:])
                    for dc in range(DC):
                        pt = ps0.tile([128, 128], F32, tag="pt")
                        nc.tensor.transpose(
                            pt[:], hn_t[:, dc * 128:(dc + 1) * 128], ident[:])
                        nc.vector.tensor_copy(
                            hnT[:, dc, t * 128:(t + 1) * 128], pt[:])

            # ---------------- S1: QT/KT (transposed + rope) and V ------------
            with (
                tc.tile_pool(name="s1t", bufs=3) as s1t,
                tc.tile_pool(name="s1c", bufs=1) as s1c,
                tc.tile_pool(name="ps1", bufs=2, space="PSUM") as ps1,
            ):
                s1w_cm = tc.tile_pool(name="s1w", bufs=3)
                s1w = s1w_cm.__enter__()
                cosT_sb = s1c.tile([HD, T], F32)
                nc.sync.dma_start(cosT_sb[:], cosT_d[:])
                sinTn_sb = s1c.tile([HD, T], F32)
                nc.sync.dma_start(sinTn_sb[:], sinTn_d[:])

                def rope_store(psum, dst, g):
                    """psum [128 d, 512 tok] -> rope -> DMA to dst [128, 512]."""
                    cs = cosT_sb[:, g * 512:(g + 1) * 512]
                    sn = sinTn_sb[:, g * 512:(g + 1) * 512]
                    raw = s1t.tile([128, 512], F32, tag="rp_raw")
                    nc.vector.tensor_copy(raw[:], psum[:])
                    rot = s1t.tile([128, 512], F32, tag="rp_rot")
                    nc.sync.dma_start(rot[0:64, :], raw[64:128, :])
                    nc.sync.dma_start(rot[64:128, :], raw[0:64, :])
                    oz = s1t.tile([128, 512], F32R, tag="rp_out")
                    nc.vector.tensor_tensor(oz[:], raw[:], cs, ALU.mult)
                    nc.vector.tensor_tensor(rot[:], rot[:], sn, ALU.mult)
                    nc.vector.tensor_tensor(oz[:], oz[:], rot[:], ALU.add)
                    nc.sync.dma_start(dst, oz[:])

                for cb in range(HQ + HKV):  # q heads then kv heads
                    is_q = cb < HQ
                    w_cb = s1w.tile([128, DC, 128], F32R, tag="w_cb")
                    _dma_split(nc, w_cb[:], wq_d[cb] if is_q else wk_d[cb - HQ], 8)
                    for g in range(TG):
                        pq = ps1.tile([128, 512], F32, tag="pq")
                        for dc in range(DC):
                            nc.tensor.matmul(
                                pq[:], w_cb[:, dc, :],
                                hnT[:, dc, g * 512:(g + 1) * 512],
                                start=(dc == 0), stop=(dc == DC - 1))
                        dst = (qt_dram[cb, :, g * 512:(g + 1) * 512] if is_q
                               else kt_dram[cb - HQ, :, g * 512:(g + 1) * 512])
                        rope_store(pq, dst, g)

                s1w_cm.__exit__(None, None, None)
                s1v_cm = tc.tile_pool(name="s1v", bufs=1)
                s1v = s1v_cm.__enter__()
                # V token-major
                wv_sb = s1v.tile([128, DC, HKV * HD], F32R)
                _dma_split(nc, wv_sb[:], wv_d[:], 8)
                for t in range(TC):
                    pv = ps1.tile([128, 256], F32, tag="pv")
                    for dc in range(DC):
                        nc.tensor.matmul(
                            pv[:], hnT[:, dc, t * 128:(t + 1) * 128],
                            wv_sb[:, dc, :],
                            start=(dc == 0), stop=(dc == DC - 1))
                    vz = s1t.tile([128, 256], F32R, tag="vz")
                    nc.vector.tensor_copy(vz[:], pv[:])
                    nc.sync.dma_start(v_dram[t * 128:(t + 1) * 128, :], vz[:])
                s1v_cm.__exit__(None, None, None)

            # ---------------- S2: attention (transposed softmax) -------------
            hnT_cm.__exit__(None, None, None)
            attnT_cm = tc.tile_pool(name="attnT_pool", bufs=1)
            attnT_pool = attnT_cm.__enter__()
            attnT = attnT_pool.tile([128, HQ, T], F32R, name="attnT")
            with (
                tc.tile_pool(name="s2c", bufs=1) as s2c,
                tc.tile_pool(name="s2t", bufs=3) as s2t,
                tc.tile_pool(name="s2sc", bufs=2) as s2sc,
                tc.tile_pool(name="ps2s", bufs=2, space="PSUM") as ps2s,
                tc.tile_pool(name="ps2o", bufs=2, space="PSUM") as ps2o,
                tc.tile_pool(name="ps2m", bufs=2, space="PSUM") as ps2m,
            ):
                kt_sb = s2c.tile([128, HKV, T], F32R)
                v_sb = s2c.tile([128, KC, HKV * HD], F32R)
                _dma_split(nc, kt_sb[:], kt_dram[:].rearrange("h p t -> p h t"), 2)
                _dma_split(nc, v_sb[:],
                           v_dram[:].rearrange("(kc p) c -> p kc c", p=128), 4)
                dmask_sb = s2c.tile([128, n_mask_tiles, 512], F32)
                nc.sync.dma_start(
                    dmask_sb[:], dmask_d[:].rearrange("n p q -> p n q"))

                for h in range(HQ):
                    kv = h // GQ
                    for g in range(TG):
                        ks = [k for k in range(KC)
                              if blocks[(g, k)][0] != "skip"]
                        qt_sb = s2t.tile([128, 512], F32R, tag="qt")
                        nc.sync.dma_start(
                            qt_sb[:], qt_dram[h, :, g * 512:(g + 1) * 512])
                        po = ps2o.tile([128, 512], F32, tag="po")
                        psum = ps2m.tile([1, 512], F32, tag="psums")
                        # pass 1: scores + tanh (one ACT table run)
                        sc_tiles = []
                        for i, k in enumerate(ks):
                            psc = ps2s.tile([128, 512], F32, tag="psc")
                            nc.tensor.matmul(
                                psc[:],
                                kt_sb[:, kv, k * 128:(k + 1) * 128],
                                qt_sb[:], start=True, stop=True)
                            sc_t = s2sc.tile([128, 512], F32, tag=f"sc{i}")
                            nc.scalar.activation(sc_t[:], psc[:], AF.Tanh,
                                                 scale=ATTN_MULT / MAX_ATTN)
                            sc_tiles.append(sc_t)
                        # pass 2: exp (one table run) + mask + PV/sums accum
                        for i, k in enumerate(ks):
                            kind, mi = blocks[(g, k)]
                            p_t = s2t.tile([128, 512], F32R, tag="pt")
                            nc.scalar.activation(p_t[:], sc_tiles[i][:],
                                                 AF.Exp, scale=MAX_ATTN,
                                                 bias=negcap_col[:])
                            if kind == "partial":
                                nc.vector.tensor_tensor(
                                    p_t[:], p_t[:], dmask_sb[:, mi, :],
                                    ALU.mult)
                            first = i == 0
                            last = i == len(ks) - 1
                            nc.tensor.matmul(
                                po[:],
                                v_sb[:, k, kv * 128:(kv + 1) * 128],
                                p_t[:], start=first, stop=last)
                            nc.tensor.matmul(
                                psum[:], ones_col[:], p_t[:],
                                start=first, stop=last)
                        recip = s2t.tile([1, 512], F32, tag="recip")
                        nc.vector.reciprocal(recip[:], psum[:])
                        pbc = ps2s.tile([128, 512], F32, tag="pbc")
                        nc.tensor.matmul(pbc[:], ones_row[:], recip[:],
                                         start=True, stop=True)
                        rb = s2t.tile([128, 512], F32, tag="rb")
                        nc.vector.tensor_copy(rb[:], pbc[:])
                        nc.vector.tensor_tensor(
                            attnT[:, h, g * 512:(g + 1) * 512], po[:], rb[:],
                            ALU.mult)

            # ---------------- S3: wo (row shard) -> partial h_attn -> AR1 ----
            with (
                tc.tile_pool(name="s3w", bufs=2) as s3w,
                tc.tile_pool(name="s3t", bufs=3) as s3t,
                tc.tile_pool(name="ps3", bufs=2, space="PSUM") as ps3,
            ):
                for db in range(DB):
                    wo_db = s3w.tile([128, HQ, 512], F32R, tag="wo_db")
                    _dma_split(nc, wo_db[:], wo_d[db], 4)
                    for t in range(TC):
                        pw = ps3.tile([128, 512], F32, tag="pw")
                        for h in range(HQ):
                            nc.tensor.matmul(
                                pw[:], attnT[:, h, t * 128:(t + 1) * 128],
                                wo_db[:, h, :],
                                start=(h == 0), stop=(h == HQ - 1))
                        oz = s3t.tile([128, 512], F32, tag="oz")
                        nc.vector.tensor_copy(oz[:], pw[:])
                        hh, dd = db // (DB // 2), db % (DB // 2)
                        nc.sync.dma_start(
                            ar1_in[hh][t * 128:(t + 1) * 128,
                                       dd * 512:(dd + 1) * 512], oz[:])
                    if db == DB // 2 - 1:
                        _ar(ar1_in[0], ar1_out[0])
            _ar(ar1_in[1], ar1_out[1])

            # ---------------- S4: h1 = x + rms(ha)*s_post; hn2T (bf16) -------
            attnT_cm.__exit__(None, None, None)
            hn2T_cm = tc.tile_pool(name="hn2T_pool", bufs=1)
            hn2T_pool = hn2T_cm.__enter__()
            hn2T = hn2T_pool.tile([128, DC, T], BF16, name="hn2T")
            with (
                tc.tile_pool(name="s4", bufs=2) as s4,
                tc.tile_pool(name="s4scr", bufs=2) as s4scr,
                tc.tile_pool(name="s4b", bufs=1) as s4b,
                tc.tile_pool(name="ps4", bufs=4, space="PSUM") as ps4,
            ):
                spa_sb = s4b.tile([128, D], F32)
                nc.sync.dma_start(spa_sb[:], spa_d[:])
                for t in range(TC):
                    ha_t = s4.tile([128, D], F32, tag="ha_t")
                    _dma_split(nc, ha_t[:, 0:DH],
                               ar1_out[0][t * 128:(t + 1) * 128, :], 2)
                    _dma_split(nc, ha_t[:, DH:D],
                               ar1_out[1][t * 128:(t + 1) * 128, :], 2)
                    x_t = s4scr.tile([128, D], F32, tag="x_t")
                    _dma_split(nc, x_t[:], x_d[t * 128:(t + 1) * 128, :], 4)
                    scr = s4scr.tile([128, D], F32, tag="scr")
                    ms_t = msp.tile([128, 1], F32, tag="ms4")
                    nc.vector.scalar_tensor_tensor(
                        scr[:], ha_t[:], 1.0, ha_t[:],
                        op0=ALU.mult, op1=ALU.mult, accum_out=ms_t[:])
                    inv_t = msp.tile([128, 1], F32, tag="inv4")
                    nc.scalar.activation(inv_t[:], ms_t[:], AF.Sqrt,
                                         bias=eps_col[:], scale=1.0 / D)
                    nc.vector.reciprocal(inv_t[:], inv_t[:])
                    h1_t = s4b.tile([128, D], F32, tag="h1_t")
                    nc.vector.scalar_tensor_tensor(
                        h1_t[:], ha_t[:], inv_t[:], spa_sb[:],
                        op0=ALU.mult, op1=ALU.mult)
                    nc.vector.tensor_tensor(h1_t[:], h1_t[:], x_t[:], ALU.add)
                    _dma_split(nc, h1_dram[t * 128:(t + 1) * 128, :].rearrange("p d -> p d"), h1_t[:], 4)
                    ms2_t = msp.tile([128, 1], F32, tag="ms4b")
                    nc.vector.scalar_tensor_tensor(
                        scr[:], h1_t[:], 1.0, h1_t[:],
                        op0=ALU.mult, op1=ALU.mult, accum_out=ms2_t[:])
                    inv2_t = msp.tile([128, 1], F32, tag="inv4b")
                    nc.scalar.activation(inv2_t[:], ms2_t[:], AF.Sqrt,
                                         bias=eps_col[:], scale=1.0 / D)
                    nc.vector.reciprocal(inv2_t[:], inv2_t[:])
                    hn2b_t = s4b.tile([128, D], BF16, tag="hn2b")
                    nc.vector.tensor_scalar_mul(hn2b_t[:], h1_t[:], inv2_t[:])
                    for dc in range(DC):
                        ptb = ps4.tile([128, 128], BF16, tag="ptb")
                        nc.tensor.transpose(
                            ptb[:], hn2b_t[:, dc * 128:(dc + 1) * 128],
                            ident_bf[:])
                        nc.vector.tensor_copy(
                            hn2T[:, dc, t * 128:(t + 1) * 128], ptb[:])

            # ---------------- S5: gateT/valT -> gelu*val -> actT (bf16) ------
            actT_cm = tc.tile_pool(name="actT_pool", bufs=1)
            actT_pool = actT_cm.__enter__()
            actT = actT_pool.tile([128, FC, T], BF16, name="actT")
            with (
                tc.tile_pool(name="s5w", bufs=2) as s5w,
                tc.tile_pool(name="s5t", bufs=2) as s5t,
                tc.tile_pool(name="ps5", bufs=4, space="PSUM") as ps5,
            ):
                for f in range(FC):
                    wg_f = s5w.tile([128, DC, 128], BF16, tag="wg_f")
                    _dma_split(nc, wg_f[:], wg_d[f], 4)
                    wv2_f = s5w.tile([128, DC, 128], BF16, tag="wv2_f")
                    _dma_split(nc, wv2_f[:], wv2_d[f], 4)
                    for g in range(TG):
                        pg = ps5.tile([128, 512], F32, tag="pg")
                        pv2 = ps5.tile([128, 512], F32, tag="pv2")
                        for dc in range(DC):
                            nc.tensor.matmul(
                                pg[:], wg_f[:, dc, :],
                                hn2T[:, dc, g * 512:(g + 1) * 512],
                                start=(dc == 0), stop=(dc == DC - 1))
                        for dc in range(DC):
                            nc.tensor.matmul(
                                pv2[:], wv2_f[:, dc, :],
                                hn2T[:, dc, g * 512:(g + 1) * 512],
                                start=(dc == 0), stop=(dc == DC - 1))
                        gel = s5t.tile([128, 512], F32, tag="gel")
                        nc.scalar.activation(gel[:], pg[:], AF.Gelu_apprx_tanh)
                        nc.vector.tensor_tensor(
                            actT[:, f, g * 512:(g + 1) * 512], gel[:], pv2[:],
                            ALU.mult)

            # ---------------- S6: w_out (row shard) -> partial h_dense -> AR2
            with (
                tc.tile_pool(name="s6w", bufs=2) as s6w,
                tc.tile_pool(name="s6t", bufs=3) as s6t,
                tc.tile_pool(name="ps6", bufs=2, space="PSUM") as ps6,
            ):
                for db in range(DB):
                    wout_db = s6w.tile([128, FC, 512], BF16, tag="wout_db")
                    _dma_split(nc, wout_db[:], wout_d[db], 4)
                    for t in range(TC):
                        pd = ps6.tile([128, 512], F32, tag="pd")
                        for f in range(FC):
                            nc.tensor.matmul(
                                pd[:], actT[:, f, t * 128:(t + 1) * 128],
                                wout_db[:, f, :],
                                start=(f == 0), stop=(f == FC - 1))
                        oz = s6t.tile([128, 512], F32, tag="oz6")
                        nc.vector.tensor_copy(oz[:], pd[:])
                        hh, dd = db // (DB // 2), db % (DB // 2)
                        nc.sync.dma_start(
                            ar2_in[hh][t * 128:(t + 1) * 128,
                                       dd * 512:(dd + 1) * 512], oz[:])
                    if db == DB // 2 - 1:
                        _ar(ar2_in[0], ar2_out[0])
            _ar(ar2_in[1], ar2_out[1])

            actT_cm.__exit__(None, None, None)
            hn2T_cm.__exit__(None, None, None)
            # ---------------- S7: out = h1 + rms(h_dense)*s_post_mlp ---------
            with (
                tc.tile_pool(name="s7", bufs=2) as s7,
                tc.tile_pool(name="s7b", bufs=1) as s7b,
            ):
                spm_sb = s7b.tile([128, D], F32)
                nc.sync.dma_start(spm_sb[:], spm_d[:])
                for t in range(TC):
                    hd_t = s7.tile([128, D], F32, tag="hd_t")
                    _dma_split(nc, hd_t[:, 0:DH],
                               ar2_out[0][t * 128:(t + 1) * 128, :], 2)
                    _dma_split(nc, hd_t[:, DH:D],
                               ar2_out[1][t * 128:(t + 1) * 128, :], 2)
                    h1_t = s7.tile([128, D], F32, tag="h1r")
                    _dma_split(nc, h1_t[:], h1_dram[t * 128:(t + 1) * 128, :], 4)
                    scr = s7.tile([128, D], F32, tag="scr7")
                    ms_t = msp.tile([128, 1], F32, tag="ms7")
                    nc.vector.scalar_tensor_tensor(
                        scr[:], hd_t[:], 1.0, hd_t[:],
                        op0=ALU.mult, op1=ALU.mult, accum_out=ms_t[:])
                    inv_t = msp.tile([128, 1], F32, tag="inv7")
                    nc.scalar.activation(inv_t[:], ms_t[:], AF.Sqrt,
                                         bias=eps_col[:], scale=1.0 / D)
                    nc.vector.reciprocal(inv_t[:], inv_t[:])
                    o_t = s7.tile([128, D], F32, tag="o_t")
                    nc.vector.scalar_tensor_tensor(
                        o_t[:], hd_t[:], inv_t[:], spm_sb[:],
                        op0=ALU.mult, op1=ALU.mult)
                    nc.vector.tensor_tensor(o_t[:], o_t[:], h1_t[:], ALU.add)
                    _dma_split(nc, out_d[t * 128:(t + 1) * 128, :], o_t[:], 4)

    nc.compile()
    return nc


_NC_CACHE = {}
_FN_CACHE = {}
LAST_RESULTS = None


def _get_sharded_fn(nc):
    """Build (once) the jitted shard_map callable for `nc` across 8 cores.

    Mirrors concourse.bass2jax.run_bass_via_pjrt's multi-core path, but caches
    the compiled function and takes pre-sharded device arrays so repeated calls
    can be timed without re-shipping inputs.
    """
    if id(nc) in _FN_CACHE:
        return _FN_CACHE[id(nc)]
    import jax
    from jax.sharding import Mesh, PartitionSpec
    from jax.experimental.shard_map import shard_map
    from concourse import bass2jax as b2j

    b2j.install_neuronx_cc_hook()
    part_name = nc.partition_id_tensor.name if nc.partition_id_tensor else None
    in_names, out_names, out_avals, zero_outs = [], [], [], []
    for alloc in nc.m.functions[0].allocations:
        if not isinstance(alloc, mybir.MemoryLocationSet):
            continue
        name = alloc.memorylocations[0].name
        if alloc.kind == "ExternalInput":
            if name == part_name:
                continue
            in_names.append(name)
        elif alloc.kind == "ExternalOutput":
            out_names.append(name)
            shape = tuple(alloc.tensor_shape)
            dtype = mybir.dt.np(alloc.dtype)
            out_avals.append(jax.core.ShapedArray(shape, dtype))
            zero_outs.append(np.zeros(shape, dtype))
    n_params = len(in_names)
    all_names = in_names + out_names
    if part_name is not None:
        all_names = all_names + [part_name]

    def _body(*args):
        operands = list(args)
        if part_name is not None:
            operands.append(b2j.partition_id_tensor())
        outs = b2j._bass_exec_p.bind(
            *operands,
            out_avals=tuple(out_avals),
            in_names=tuple(all_names),
            out_names=tuple(out_names),
            lowering_input_output_aliases=(),
            sim_require_finite=True,
            sim_require_nnan=True,
            nc=nc,
        )
        return tuple(outs)

    devices = jax.devices()[:N_CORES]
    mesh = Mesh(np.asarray(devices), ("core",))
    n_outs = len(out_names)
    donate = tuple(range(n_params, n_params + n_outs))
    sharded = jax.jit(
        shard_map(
            _body,
            mesh=mesh,
            in_specs=(PartitionSpec("core"),) * (n_params + n_outs),
            out_specs=(PartitionSpec("core"),) * n_outs,
            check_rep=False,
        ),
        donate_argnums=donate,
        keep_unused=True,
    )
    entry = dict(
        fn=sharded, in_names=in_names, out_names=out_names,
        out_avals=out_avals, zero_outs=zero_outs, mesh=mesh,
    )
    _FN_CACHE[id(nc)] = entry
    return entry


def _device_inputs(nc, in_maps):
    import jax
    from jax.sharding import NamedSharding, PartitionSpec

    entry = _get_sharded_fn(nc)
    sh = NamedSharding(entry["mesh"], PartitionSpec("core"))
    concat_in = [
        np.concatenate([np.asarray(m[name]) for m in in_maps], axis=0)
        for name in entry["in_names"]
    ]
    return [jax.device_put(a, sh) for a in concat_in]


def _dev_zeros(nc):
    import jax
    from jax.sharding import NamedSharding, PartitionSpec

    entry = _get_sharded_fn(nc)
    sh = NamedSharding(entry["mesh"], PartitionSpec("core"))
    return [
        jax.device_put(
            np.zeros((N_CORES * z.shape[0], *z.shape[1:]), z.dtype), sh)
        for z in entry["zero_outs"]
    ]


def _run(nc, dev_in):
    entry = _get_sharded_fn(nc)
    out_arrs = entry["fn"](*dev_in, *_dev_zeros(nc))
    outs = []
    for i, name in enumerate(entry["out_names"]):
        shp = entry["out_avals"][i].shape
        outs.append(np.asarray(out_arrs[i]).reshape(N_CORES, *shp))
    return dict(zip(entry["out_names"], outs))


def _run_timed(nc, dev_in, iters=5):
    """Returns (per-call wall seconds list). Inputs already device-resident;
    donated zero buffers are re-staged outside the timed window."""
    import time as _time

    entry = _get_sharded_fn(nc)
    times = []
    for _ in range(iters):
        zeros = _dev_zeros(nc)
        for z in zeros:
            z.block_until_ready()
        t0 = _time.perf_counter()
        out = entry["fn"](*dev_in, *zeros)
        for o in out:
            o.block_until_ready()
        times.append(_time.perf_counter() - t0)
    return times


def _run_timed_pipelined(nc, dev_in, iters=8):
    """Enqueue `iters` executions back-to-back (async dispatch), block once.
    Returns (total_s, per_iter_slope_s) where slope excludes one-time overhead:
    slope = (t_N - t_1) / (N - 1)."""
    import time as _time

    entry = _get_sharded_fn(nc)
    zsets = [_dev_zeros(nc) for _ in range(iters)]
    for zs in zsets:
        for z in zs:
            z.block_until_ready()
    # one warm call
    out = entry["fn"](*dev_in, *_dev_zeros(nc))
    for o in out:
        o.block_until_ready()

    t0 = _time.perf_counter()
    out = entry["fn"](*dev_in, *zsets[0])
    for o in out:
        o.block_until_ready()
    t1 = _time.perf_counter()

    outs = []
    for i in range(1, iters):
        outs.append(entry["fn"](*dev_in, *zsets[i]))
    for os_ in outs:
        for o in os_:
            o.block_until_ready()
    t2 = _time.perf_counter()
    one = t1 - t0
    slope = (t2 - t1) / (iters - 1) if iters > 1 else one
    return one, slope


def _arr_qk(w, nchunks):
    # [D, nchunks*128] -> [nchunks, 128(part), DC, 128]; fully contiguous DMA
    d, c = w.shape
    return np.ascontiguousarray(
        w.reshape(d // 128, 128, nchunks, 128).transpose(2, 1, 0, 3))


def _arr_v(w):
    # [D, C] -> [128(part), DC, C]
    d, c = w.shape
    return np.ascontiguousarray(w.reshape(d // 128, 128, c).transpose(1, 0, 2))


def _arr_o(w, nchunks):
    # [nchunks*128, D] -> [DB, 128(part), nchunks, 512]
    r, d = w.shape
    return np.ascontiguousarray(
        w.reshape(nchunks, 128, d // 512, 512).transpose(2, 1, 0, 3))


def _prepare(inputs):
    x = np.asarray(inputs["x"], np.float32)
    mask_qk = np.asarray(inputs["mask"]).reshape(S, S).astype(bool)
    s_pre_attn = np.asarray(inputs["scale_pre_attn"], np.float32)
    s_post_attn = np.asarray(inputs["scale_post_attn"], np.float32)
    s_pre_mlp = np.asarray(inputs["scale_pre_mlp"], np.float32)
    s_post_mlp = np.asarray(inputs["scale_post_mlp"], np.float32)
    wq = np.asarray(inputs["wq"], np.float32) * s_pre_attn[:, None]
    wk = np.asarray(inputs["wk"], np.float32) * s_pre_attn[:, None]
    wv = np.asarray(inputs["wv"], np.float32) * s_pre_attn[:, None]
    wo = np.asarray(inputs["wo"], np.float32)
    wg = np.asarray(inputs["w_gate"], np.float32) * s_pre_mlp[:, None]
    wv2 = np.asarray(inputs["w_val"], np.float32) * s_pre_mlp[:, None]
    wout = np.asarray(inputs["w_out"], np.float32)

    blocks, dmask = _analyze_mask(mask_qk)
    key = tuple(sorted((k, v[0], v[1]) for k, v in blocks.items()))
    if key not in _NC_CACHE:
        _NC_CACHE[key] = _build_nc(blocks, dmask.shape[0])
    nc = _NC_CACHE[key]

    # FFN zero-padding to a multiple of 512 (22*128 per TP rank)
    wg_p = np.zeros((D, FFN_PAD), ml_dtypes.bfloat16)
    wg_p[:, :FFN] = wg.astype(ml_dtypes.bfloat16)
    wv2_p = np.zeros((D, FFN_PAD), ml_dtypes.bfloat16)
    wv2_p[:, :FFN] = wv2.astype(ml_dtypes.bfloat16)
    wout_p = np.zeros((FFN_PAD, D), ml_dtypes.bfloat16)
    wout_p[:FFN, :] = wout.astype(ml_dtypes.bfloat16)

    # RoPE tables in T-layout
    inv_freq = 1.0 / (BASE ** (np.arange(0, HD, 2, dtype=np.float64) / HD))
    phase = np.arange(S, dtype=np.float64)[:, None] * inv_freq[None, :]
    cos_f = np.cos(phase).astype(np.float32)   # [S, 64]
    sin_f = np.sin(phase).astype(np.float32)
    cosT = np.concatenate([cos_f.T, cos_f.T], axis=0)           # [128, S]
    sinTn = np.concatenate([-sin_f.T, sin_f.T], axis=0)         # [128, S]

    spa_bc = np.ascontiguousarray(
        np.broadcast_to(s_post_attn, (128, D)), dtype=np.float32)
    spm_bc = np.ascontiguousarray(
        np.broadcast_to(s_post_mlp, (128, D)), dtype=np.float32)

    in_maps = []
    for c in range(N_CORES):
        b, m = c // TP, c % TP
        in_maps.append({
            "x": np.ascontiguousarray(x[b]),
            "wq": _arr_qk(wq[:, m * HQ * HD:(m + 1) * HQ * HD], HQ),
            "wk": _arr_qk(wk[:, m * HKV * HD:(m + 1) * HKV * HD], HKV),
            "wv": _arr_v(wv[:, m * HKV * HD:(m + 1) * HKV * HD]),
            "wo": _arr_o(wo[m * HQ * HD:(m + 1) * HQ * HD, :], HQ),
            "wg": _arr_qk(wg_p[:, m * F:(m + 1) * F], FC),
            "wv2": _arr_qk(wv2_p[:, m * F:(m + 1) * F], FC),
            "wout": _arr_o(wout_p[m * F:(m + 1) * F, :], FC),
            "sp_attn": spa_bc,
            "sp_mlp": spm_bc,
            "cosT": cosT,
            "sinTn": sinTn,
            "dmask": dmask,
        })

    return nc, in_maps


def kernel(**inputs):
    global LAST_RESULTS
    nc, in_maps = _prepare(inputs)
    from concourse._compat import axon_active
    if axon_active():
        # axon client: cached jit/shard_map path (run_bass_kernel_spmd would
        # also work but re-ships inputs per call)
        dev_in = _device_inputs(nc, in_maps)
        res = _run(nc, dev_in)
        LAST_RESULTS = res
        out = np.stack([res["out"][0], res["out"][TP]])
    else:
        # native path (real /dev/neuron*): NRT execution, NTFF-capable
        r = run_bass_kernel_spmd(nc, in_maps, core_ids=list(range(N_CORES)))
        LAST_RESULTS = r
        out = np.stack([r.results[0]["out"], r.results[TP]["out"]])
    return out.astype(np.float32)
